# revision 64
# baseline (speedup 1.0000x reference)
"""Class-conditional linear dispatch (MoE routing) on 8 trn2 NeuronCores.

y[i] = x[i] @ W[cls[i]] + b[cls[i]]   with B=8192, D=512, C=16 classes.

Strategy: expert-parallel with host-side dispatch. The host computes the
routing from `cls` (argsort by class), assigns two classes per core --
pairing large classes with small ones so the per-core tile count is
minimal -- converts x/W to bf16, and pre-packs each core's rows as the
exact SBUF images the kernel wants (K-major, chunked), so every device
load is a plain HWDGE DMA with 128 contiguous per-partition runs.

On device, per 128-row tile: 4 accumulating bf16 matmuls against the
class's K-chunked weights (f32 PSUM), then a DVE/ACT copy (alternating,
so the two engines work in parallel) casts to bf16 into a per-class
output block that is stored as one large DMA per class (alternating
SP/ACT rings). The host scatters the per-core bf16 outputs back to
original row order, adding the (f32) bias during the scatter.

The For_i timing loop uses staggered_reset and emits `reps` independent
problem instances per loop body (loop_reps keeps meaning instances:
trip count = ceil(loop_reps/reps)). The per-iteration all-engine
barrier amortizes over the instances and the Tile scheduler overlaps
instance i+1's loads with instance i's compute inside one body --
measured 17.5us -> 11.7us per instance going from reps=1 to reps=6.
The design also minimizes total DMA bytes (all-bf16: 2.125 MB in +
1.125 MB out per core), DMA count (large contiguous host-prepacked
images), and PE work (36 matmuls; classes paired big+small so per-core
capacity is 640+512 rows instead of 640+640).
"""

import sys

import numpy as np

_TRN_REPO = "/opt/trn_rl_repo"
if _TRN_REPO not in sys.path:
    sys.path.insert(0, _TRN_REPO)

B, D_IN, D_OUT, C, NCORES = 8192, 512, 512, 16, 8
CPL = C // NCORES  # class slots per core
KC = D_IN // 128  # contraction chunks of 128

# Set by callers that want profiling; results stashed in LAST_RESULT.
TRACE = False
LAST_RESULT = None


def plan_chunks(n_tiles, gather_chunk, first_small, c, last_small=False):
    """Row-tile chunk sizes for class-slot c's loads (host & device agree)."""
    chunks, rem = [], n_tiles
    tail = 1 if last_small and c == CPL - 1 and n_tiles > 1 else 0
    rem -= tail
    if first_small and c == 0 and rem > 1:
        chunks.append(1)
        rem -= 1
    while rem > 0:
        gc = min(gather_chunk, rem)
        chunks.append(gc)
        rem -= gc
    if tail:
        chunks.append(1)
    return chunks


def plan_interleave(n_tiles):
    """Class-interleaved load + compute order (host & device agree).

    Loads: x c0 (2 tiles), W c0, x c1 (2 tiles), W c1, then alternating
    2-tile x chunks. Compute consumes tiles in the same arrival order, so
    the PE starts after ~768KB and never starves while later chunks and
    the stores share the remaining DMA stream.
    """
    load_ops = []  # ("x", c, t0, gc) | ("w", c)
    pos = [0] * CPL
    for c in range(CPL):
        gc = min(2, n_tiles[c])
        load_ops.append(("x", c, 0, gc))
        load_ops.append(("w", c))
        pos[c] = gc
    turn = 0
    while any(pos[c] < n_tiles[c] for c in range(CPL)):
        c = turn % CPL
        if pos[c] < n_tiles[c]:
            gc = min(2, n_tiles[c] - pos[c])
            load_ops.append(("x", c, pos[c], gc))
            pos[c] += gc
        turn += 1
    tile_order = []
    tp = [0] * CPL
    turn = 0
    while any(tp[c] < n_tiles[c] for c in range(CPL)):
        c = turn % CPL
        n = 0
        while tp[c] < n_tiles[c] and n < 2:
            tile_order.append((c, tp[c]))
            tp[c] += 1
            n += 1
        turn += 1
    return load_ops, tile_order


def build_nc(
    caps,
    *,
    gather_chunk: int = 3,
    first_small: bool = True,
    last_small: bool = False,
    y_f32: bool = False,
    w_eng: str = "scalar",
    w_batch: bool = True,
    w_first: bool = False,
    load_alt: bool = False,
    store_eng: str = "sync",
    out_batch: bool = False,
    copy_split: bool = False,
    gbufs: int = 4,
    wbufs: int = 1,
    psum_bufs: int = 3,
    ybufs: int = 4,
    staggered: bool = False,
    warm_mms: bool = False,
    pipelined: bool = False,
    interleave: bool = False,
    unroll: int = 4,
    reps: int = 1,
    loop_reps: int = 1,
    no_compute: bool = False,
    no_loads: bool = False,
    no_stores: bool = False,
    detach: bool = False,
):
    """Per-core Bass program. caps = rows capacity per class slot
    (each a multiple of 128); int means uniform."""
    import concourse.bacc as bacc
    import concourse.mybir as mybir
    from concourse import tile

    f32 = mybir.dt.float32
    bf16 = mybir.dt.bfloat16
    if isinstance(caps, int):
        caps = (caps,) * CPL
    caps = tuple(int(v) for v in caps)
    n_tiles = [v // 128 for v in caps]
    base = [sum(caps[:c]) for c in range(CPL)]
    r_cap = sum(caps)
    x_cols = KC * r_cap

    nc = bacc.Bacc("TRN2", target_bir_lowering=False, debug=False)
    x_d = nc.dram_tensor("xp", [128, x_cols], bf16, kind="ExternalInput")
    w_d = nc.dram_tensor("wl", [CPL, 128, KC * D_OUT], bf16, kind="ExternalInput")
    y_dt = f32 if y_f32 else bf16
    y_d = nc.dram_tensor("y", [r_cap, D_OUT], y_dt, kind="ExternalOutput")

    with tile.TileContext(nc) as tc:
        from contextlib import ExitStack, nullcontext

        if interleave:
            load_ops, tile_order = plan_interleave(n_tiles)
            with (
                tc.tile_pool(name="wpool", bufs=wbufs) as wpool,
                tc.tile_pool(name="xin", bufs=gbufs) as gpool,
                tc.tile_pool(name="yout", bufs=ybufs) as ypool,
                tc.tile_pool(name="psy", bufs=psum_bufs, space="PSUM") as psyp,
                tc.For_i(0, loop_reps, 1, staggered_reset=staggered)
                if loop_reps > 1
                else nullcontext(),
            ):
                w_sb = wpool.tile([128, CPL * KC, D_OUT], bf16)
                tile_src = {}
                x_off = 0
                for op in load_ops:
                    if op[0] == "x":
                        _, c, t0, gc = op
                        g = gpool.tile([128, KC, gc * 128], bf16)
                        width = KC * gc * 128
                        nc.sync.dma_start(g[:], x_d[:, x_off : x_off + width])
                        x_off += width
                        for j in range(gc):
                            tile_src[(c, t0 + j)] = (g, j * 128)
                    else:
                        c = op[1]
                        if w_batch:
                            nc.scalar.dma_start(
                                w_sb[:, c * KC : (c + 1) * KC, :], w_d[c]
                            )
                        else:
                            for k in range(KC):
                                nc.scalar.dma_start(
                                    w_sb[:, c * KC + k, :],
                                    w_d[c, :, k * D_OUT : (k + 1) * D_OUT],
                                )

                for ti, (c, t) in enumerate(tile_order):
                    g, lo = tile_src[(c, t)]
                    y_ps = psyp.tile([128, D_OUT], f32)
                    for k in range(KC):
                        nc.tensor.matmul(
                            y_ps[:],
                            g[:, k, lo : lo + 128],
                            w_sb[:, c * KC + k, :],
                            start=(k == 0),
                            stop=(k == KC - 1),
                        )
                    y_sb = ypool.tile([128, D_OUT], y_dt)
                    if copy_split and ti % 2:
                        nc.scalar.copy(y_sb[:], y_ps[:])
                    else:
                        nc.vector.tensor_copy(y_sb[:], y_ps[:])
                    row0 = base[c] + t * 128
                    eng = nc.sync if ti % 2 == 0 else nc.scalar
                    eng.dma_start(y_d[row0 : row0 + 128, :], y_sb[:])
        elif pipelined and loop_reps > 1:
            # software-pipelined timing loop: load[i+2] || compute[i+1] ||
            # store[i]; the all-engine barrier amortizes over `unroll` ticks
            chunk_plan = []
            xo = 0
            for c in range(CPL):
                t0 = 0
                for gc in plan_chunks(
                    n_tiles[c], gather_chunk, first_small, c, last_small
                ):
                    chunk_plan.append((c, t0, gc, xo))
                    xo += KC * gc * 128
                    t0 += gc

            with tc.tile_pool(name="psy", bufs=psum_bufs, space="PSUM") as psyp:

                def st_load(pipe, iv):
                    w_t = pipe.intermediate_tile([128, CPL * KC, D_OUT], bf16)
                    gs = []
                    for c, t0, gc, off in chunk_plan:
                        g = pipe.intermediate_tile([128, KC, gc * 128], bf16)
                        nc.sync.dma_start(g[:], x_d[:, off : off + KC * gc * 128])
                        gs.append(g)
                    for c in range(CPL):
                        nc.scalar.dma_start(
                            w_t[:, c * KC : (c + 1) * KC, :], w_d[c]
                        )
                    return (w_t, *gs)

                def st_compute(pipe, iv, tiles):
                    w_t = tiles[0]
                    gs = tiles[1:]
                    tsrc = {}
                    for gi, (c, t0, gc, off) in enumerate(chunk_plan):
                        for j in range(gc):
                            tsrc[(c, t0 + j)] = (gs[gi], j * 128)
                    ybigs = []
                    ti = 0
                    for c in range(CPL):
                        y_big = pipe.intermediate_tile(
                            [128, n_tiles[c], D_OUT], y_dt
                        )
                        for t in range(n_tiles[c]):
                            g, lo = tsrc[(c, t)]
                            y_ps = psyp.tile([128, D_OUT], f32)
                            for k in range(KC):
                                nc.tensor.matmul(
                                    y_ps[:],
                                    g[:, k, lo : lo + 128],
                                    w_t[:, c * KC + k, :],
                                    start=(k == 0),
                                    stop=(k == KC - 1),
                                )
                            if copy_split and ti % 2:
                                nc.scalar.copy(y_big[:, t, :], y_ps[:])
                            else:
                                nc.vector.tensor_copy(y_big[:, t, :], y_ps[:])
                            ti += 1
                        ybigs.append(y_big)
                    return tuple(ybigs)

                def st_store(pipe, iv, ybigs):
                    for c, y_big in enumerate(ybigs):
                        eng = nc.sync if c % 2 == 0 else nc.scalar
                        eng.dma_start(
                            y_d[base[c] : base[c] + caps[c], :].rearrange(
                                "(t p) n -> p t n", p=128
                            ),
                            y_big[:],
                        )

                tc.For_i_pipelined(
                    [st_load, st_compute, st_store],
                    0,
                    loop_reps,
                    unroll=unroll,
                )
        else:
            _build_sequential = True
        if not (interleave or (pipelined and loop_reps > 1)):
          eff_reps = reps if loop_reps > 1 else 1
          trips = (loop_reps + eff_reps - 1) // eff_reps
          with (
            tc.tile_pool(name="wpool", bufs=wbufs) as wpool,
            tc.tile_pool(name="xin", bufs=gbufs) as gpool,
            tc.tile_pool(name="yout", bufs=ybufs) as ypool,
            tc.tile_pool(name="psy", bufs=psum_bufs, space="PSUM") as psyp,
            tc.For_i(0, trips, 1, staggered_reset=staggered)
            if loop_reps > 1
            else nullcontext(),
          ):
           for _rep in range(eff_reps):
            w_sb = wpool.tile([128, CPL * KC, D_OUT], bf16)
            w_dma = nc.scalar if w_eng == "scalar" else nc.sync
            tile_src = {}  # (c, t) -> (x tile, col offset)
            n_load = 0
            x_off = 0
            if no_loads or detach:
                tiny = gpool.tile([128, 8], bf16)
                nc.sync.dma_start(tiny[:], x_d[:, 0:8])
                for c in range(CPL):
                    g = gpool.tile([128, KC, n_tiles[c] * 128], bf16)
                    nc.vector.memset(g[:, 0, 0:8], 0.0)
                    for t in range(n_tiles[c]):
                        tile_src[(c, t)] = (g, t * 128)
                if not no_compute:
                    nc.vector.memset(w_sb[:, 0, 0:8], 0.0)
            detached = {}
            if warm_mms:
                warm_ps = psyp.tile([128, 64], f32, tag="warmps")

            def warm_kick(src_ap):
                # keep the PE HAM activity window non-idle during the load
                # phase: a ~80ns matmul chained to each arriving chunk
                if warm_mms:
                    nc.tensor.matmul(
                        warm_ps[:],
                        src_ap,
                        src_ap[:, 0:64],
                        start=True,
                        stop=True,
                        skip_group_check=True,
                    )

            for c in range(CPL if not no_loads else 0):

                def load_w(c=c):
                    # W[c] K-chunked SBUF image: [128, KC, D_OUT]
                    if detach:
                        w_dst = wpool.tile([128, KC, D_OUT], bf16, tag="wdump")
                    else:
                        w_dst = w_sb[:, c * KC : (c + 1) * KC, :]
                    if w_batch:
                        w_dma.dma_start(w_dst[:] if detach else w_dst, w_d[c])
                    else:
                        for k in range(KC):
                            w_dma.dma_start(
                                w_dst[:, k, :]
                                if detach
                                else w_sb[:, c * KC + k, :],
                                w_d[c, :, k * D_OUT : (k + 1) * D_OUT],
                            )

                if w_first:
                    load_w()
                t0 = 0
                for gc in plan_chunks(n_tiles[c], gather_chunk, first_small, c, last_small):
                    g = gpool.tile([128, KC, gc * 128], bf16)
                    width = KC * gc * 128
                    x_dma = (
                        (nc.sync if n_load % 2 == 0 else nc.scalar)
                        if load_alt
                        else nc.sync
                    )
                    x_dma.dma_start(g[:], x_d[:, x_off : x_off + width])
                    x_off += width
                    warm_kick(g[:, 0, 0:128])
                    for j in range(gc):
                        if detach:
                            detached[(c, t0 + j)] = (g, j * 128)
                        else:
                            tile_src[(c, t0 + j)] = (g, j * 128)
                    t0 += gc
                    n_load += 1
                if not w_first:
                    load_w()

            # bias is folded in on the host during the scatter-back
            def pick_store(i):
                if store_eng == "alt":
                    return nc.sync if i % 2 == 0 else nc.scalar
                return nc.scalar if store_eng == "scalar" else nc.sync

            ti = 0
            for c in range(CPL if not no_compute else 0):
                if out_batch:
                    y_big = ypool.tile([128, n_tiles[c], D_OUT], y_dt)
                for t in range(n_tiles[c]):
                    g, lo = tile_src[(c, t)]
                    y_ps = psyp.tile([128, D_OUT], f32)
                    for k in range(KC):
                        nc.tensor.matmul(
                            y_ps[:],
                            g[:, k, lo : lo + 128],
                            w_sb[:, c * KC + k, :],
                            start=(k == 0),
                            stop=(k == KC - 1),
                        )
                    use_act = copy_split and ti % 2

                    def cp(dst, src):
                        if use_act:
                            nc.scalar.copy(dst, src)
                        else:
                            nc.vector.tensor_copy(dst, src)

                    if out_batch:
                        cp(y_big[:, t, :], y_ps[:])
                    else:
                        y_sb = ypool.tile([128, D_OUT], y_dt)
                        cp(y_sb[:], y_ps[:])
                        row0 = base[c] + t * 128
                        if not no_stores:
                            pick_store(ti).dma_start(
                                y_d[row0 : row0 + 128, :], y_sb[:]
                            )
                    ti += 1
                if out_batch and not no_stores:
                    pick_store(c).dma_start(
                        y_d[base[c] : base[c] + caps[c], :].rearrange(
                            "(t p) n -> p t n", p=128
                        ),
                        y_big[:],
                    )

    nc.compile()
    return nc


def _route(cls_np: np.ndarray):
    """Routing: per-class row lists, class->slot assignment, slot capacities.

    Classes are sorted by row count; the 8 largest go to slot 0 (one per
    core), the 8 smallest to slot 1, so the shared per-slot capacity
    (max over cores, rounded up to 128) is minimal.
    """
    order = np.argsort(cls_np, kind="stable")
    counts = np.bincount(cls_np, minlength=C)
    starts = np.zeros(C + 1, dtype=np.int64)
    starts[1:] = np.cumsum(counts)
    rows_per_class = [order[starts[c] : starts[c + 1]] for c in range(C)]

    by_size = sorted(range(C), key=lambda c: -counts[c])
    perm = [0] * C  # global slot g = core*CPL + j -> original class id
    for k in range(NCORES):
        perm[k * CPL] = by_size[k]  # big classes in slot 0
        perm[k * CPL + 1] = by_size[C - 1 - k]  # small classes in slot 1
    cap = [0] * CPL
    for j in range(CPL):
        mx = max(counts[perm[k * CPL + j]] for k in range(NCORES))
        cap[j] = max(128, -(-int(mx) // 128) * 128)
    return rows_per_class, perm, tuple(cap)


# Variant shipped by kernel(); exp.py/bench.py sweep alternatives.
BEST_VARIANT = {
    "gather_chunk": 2,
    "first_small": True,
    "out_batch": True,
    "store_eng": "alt",
    "copy_split": True,
    "psum_bufs": 8,
    "wbufs": 6,
    "gbufs": 30,
    "ybufs": 18,
    "staggered": True,
    "reps": 6,
}


def make_in_maps(x, rows_per_class, perm, caps, W, b, **variant):
    """Per-core input maps matching build_nc(caps, **variant)."""
    import concourse.mybir as mybir

    bf16 = mybir.dt.np(mybir.dt.bfloat16)
    n_tiles = [v // 128 for v in caps]
    gather_chunk = variant.get("gather_chunk", 3)
    first_small = variant.get("first_small", True)
    last_small = variant.get("last_small", False)
    interleave = variant.get("interleave", False)
    x_bf = np.ascontiguousarray(np.asarray(x, dtype=np.float32).astype(bf16))
    W_bf = np.asarray(W, dtype=np.float32).astype(bf16)

    def img(blk, gc):
        # SBUF image [128, KC, gc*128]: g[p,kk,r] = blk[r, kk*128+p]
        return (
            blk.reshape(gc * 128, KC, 128)
            .transpose(2, 1, 0)
            .reshape(128, KC * gc * 128)
        )

    in_maps = []
    for k in range(NCORES):
        xcs = []
        wls = []
        for j in range(CPL):
            c = perm[k * CPL + j]
            rows = rows_per_class[c]
            idx = np.zeros(caps[j], dtype=np.int64)
            idx[: len(rows)] = rows
            xcs.append(x_bf[idx])  # [caps[j], D_IN]
            # W image [128, KC*D_OUT]: w[p, kk*D+n] = W[c, kk*128+p, n]
            wls.append(
                W_bf[c]
                .reshape(KC, 128, D_OUT)
                .transpose(1, 0, 2)
                .reshape(128, KC * D_OUT)
            )
        cols = []
        if interleave:
            for op in plan_interleave(n_tiles)[0]:
                if op[0] != "x":
                    continue
                _, j, t0, gc = op
                cols.append(img(xcs[j][t0 * 128 : (t0 + gc) * 128], gc))
        else:
            for j in range(CPL):
                t0 = 0
                for gc in plan_chunks(
                    n_tiles[j], gather_chunk, first_small, j, last_small
                ):
                    cols.append(img(xcs[j][t0 * 128 : (t0 + gc) * 128], gc))
                    t0 += gc
        in_maps.append(
            {
                "xp": np.ascontiguousarray(np.concatenate(cols, axis=1)),
                "wl": np.ascontiguousarray(np.stack(wls)),
            }
        )
    return in_maps


def kernel(x, cls, W, b):
    from concourse.bass_utils import run_bass_kernel_spmd

    global LAST_RESULT
    cls_np = np.asarray(cls).astype(np.int64).ravel()

    rows_per_class, perm, caps = _route(cls_np)

    in_maps = make_in_maps(x, rows_per_class, perm, caps, W, b, **BEST_VARIANT)
    nc = build_nc(caps, **BEST_VARIANT)
    res = run_bass_kernel_spmd(
        nc,
        in_maps,
        core_ids=list(range(NCORES)),
        trace=TRACE,
        trace_cores=list(range(NCORES)) if TRACE else None,
    )
    LAST_RESULT = res

    b_np = np.asarray(b, dtype=np.float32)
    base = [sum(caps[:j]) for j in range(CPL)]
    out = np.empty((B, D_OUT), dtype=np.float32)
    for k in range(NCORES):
        y = np.asarray(res.results[k]["y"]).astype(np.float32)
        for j in range(CPL):
            c = perm[k * CPL + j]
            rows = rows_per_class[c]
            out[rows] = y[base[j] : base[j] + len(rows)] + b_np[c]
    return out


# revision 67
# speedup vs baseline: 1.0210x; 1.0210x over previous
"""Class-conditional linear dispatch (MoE routing) on 8 trn2 NeuronCores.

y[i] = x[i] @ W[cls[i]] + b[cls[i]]   with B=8192, D=512, C=16 classes.

Strategy: expert-parallel with host-side dispatch. The host computes the
routing from `cls` (argsort by class), assigns two classes per core --
pairing large classes with small ones so the per-core tile count is
minimal -- converts x/W to bf16, and pre-packs each core's rows as the
exact SBUF images the kernel wants (K-major, chunked), so every device
load is a plain HWDGE DMA with 128 contiguous per-partition runs.

On device, per 128-row tile: 4 accumulating bf16 matmuls against the
class's K-chunked weights (f32 PSUM), then a DVE/ACT copy (alternating,
so the two engines work in parallel) casts to bf16 into a per-class
output block that is stored as one large DMA per class (alternating
SP/ACT rings). The host scatters the per-core bf16 outputs back to
original row order, adding the (f32) bias during the scatter.

The For_i timing loop uses staggered_reset and emits `reps` independent
problem instances per loop body (loop_reps keeps meaning instances:
trip count = ceil(loop_reps/reps)). The per-iteration all-engine
barrier amortizes over the instances and the Tile scheduler overlaps
instance i+1's loads with instance i's compute inside one body --
measured 17.5us -> 11.7us per instance going from reps=1 to reps=6.
The design also minimizes total DMA bytes (all-bf16: 2.125 MB in +
1.125 MB out per core), DMA count (large contiguous host-prepacked
images), and PE work (36 matmuls; classes paired big+small so per-core
capacity is 640+512 rows instead of 640+640).
"""

import sys

import numpy as np

_TRN_REPO = "/opt/trn_rl_repo"
if _TRN_REPO not in sys.path:
    sys.path.insert(0, _TRN_REPO)

B, D_IN, D_OUT, C, NCORES = 8192, 512, 512, 16, 8
CPL = C // NCORES  # class slots per core
KC = D_IN // 128  # contraction chunks of 128

# Set by callers that want profiling; results stashed in LAST_RESULT.
TRACE = False
LAST_RESULT = None


def plan_chunks(n_tiles, gather_chunk, first_small, c, last_small=False):
    """Row-tile chunk sizes for class-slot c's loads (host & device agree)."""
    chunks, rem = [], n_tiles
    tail = 1 if last_small and c == CPL - 1 and n_tiles > 1 else 0
    rem -= tail
    if first_small and c == 0 and rem > 1:
        chunks.append(1)
        rem -= 1
    while rem > 0:
        gc = min(gather_chunk, rem)
        chunks.append(gc)
        rem -= gc
    if tail:
        chunks.append(1)
    return chunks


def plan_interleave(n_tiles):
    """Class-interleaved load + compute order (host & device agree).

    Loads: x c0 (2 tiles), W c0, x c1 (2 tiles), W c1, then alternating
    2-tile x chunks. Compute consumes tiles in the same arrival order, so
    the PE starts after ~768KB and never starves while later chunks and
    the stores share the remaining DMA stream.
    """
    load_ops = []  # ("x", c, t0, gc) | ("w", c)
    pos = [0] * CPL
    for c in range(CPL):
        gc = min(2, n_tiles[c])
        load_ops.append(("x", c, 0, gc))
        load_ops.append(("w", c))
        pos[c] = gc
    turn = 0
    while any(pos[c] < n_tiles[c] for c in range(CPL)):
        c = turn % CPL
        if pos[c] < n_tiles[c]:
            gc = min(2, n_tiles[c] - pos[c])
            load_ops.append(("x", c, pos[c], gc))
            pos[c] += gc
        turn += 1
    tile_order = []
    tp = [0] * CPL
    turn = 0
    while any(tp[c] < n_tiles[c] for c in range(CPL)):
        c = turn % CPL
        n = 0
        while tp[c] < n_tiles[c] and n < 2:
            tile_order.append((c, tp[c]))
            tp[c] += 1
            n += 1
        turn += 1
    return load_ops, tile_order


def build_nc(
    caps,
    *,
    gather_chunk: int = 3,
    first_small: bool = True,
    last_small: bool = False,
    y_f32: bool = False,
    w_eng: str = "scalar",
    w_batch: bool = True,
    w_first: bool = False,
    load_alt: bool = False,
    store_eng: str = "sync",
    out_batch: bool = False,
    copy_split: bool = False,
    gbufs: int = 4,
    wbufs: int = 1,
    psum_bufs: int = 3,
    ybufs: int = 4,
    staggered: bool = False,
    warm_mms: bool = False,
    pipelined: bool = False,
    interleave: bool = False,
    unroll: int = 4,
    reps: int = 1,
    loop_reps: int = 1,
    no_compute: bool = False,
    no_loads: bool = False,
    no_stores: bool = False,
    detach: bool = False,
):
    """Per-core Bass program. caps = rows capacity per class slot
    (each a multiple of 128); int means uniform."""
    import concourse.bacc as bacc
    import concourse.mybir as mybir
    from concourse import tile

    f32 = mybir.dt.float32
    bf16 = mybir.dt.bfloat16
    if isinstance(caps, int):
        caps = (caps,) * CPL
    caps = tuple(int(v) for v in caps)
    n_tiles = [v // 128 for v in caps]
    base = [sum(caps[:c]) for c in range(CPL)]
    r_cap = sum(caps)
    x_cols = KC * r_cap

    nc = bacc.Bacc("TRN2", target_bir_lowering=False, debug=False)
    x_d = nc.dram_tensor("xp", [128, x_cols], bf16, kind="ExternalInput")
    w_d = nc.dram_tensor("wl", [CPL, 128, KC * D_OUT], bf16, kind="ExternalInput")
    y_dt = f32 if y_f32 else bf16
    y_d = nc.dram_tensor("y", [r_cap, D_OUT], y_dt, kind="ExternalOutput")

    with tile.TileContext(nc) as tc:
        from contextlib import ExitStack, nullcontext

        if interleave:
            load_ops, tile_order = plan_interleave(n_tiles)
            with (
                tc.tile_pool(name="wpool", bufs=wbufs) as wpool,
                tc.tile_pool(name="xin", bufs=gbufs) as gpool,
                tc.tile_pool(name="yout", bufs=ybufs) as ypool,
                tc.tile_pool(name="psy", bufs=psum_bufs, space="PSUM") as psyp,
                tc.For_i(0, loop_reps, 1, staggered_reset=staggered)
                if loop_reps > 1
                else nullcontext(),
            ):
                w_sb = wpool.tile([128, CPL * KC, D_OUT], bf16)
                tile_src = {}
                x_off = 0
                for op in load_ops:
                    if op[0] == "x":
                        _, c, t0, gc = op
                        g = gpool.tile([128, KC, gc * 128], bf16)
                        width = KC * gc * 128
                        nc.sync.dma_start(g[:], x_d[:, x_off : x_off + width])
                        x_off += width
                        for j in range(gc):
                            tile_src[(c, t0 + j)] = (g, j * 128)
                    else:
                        c = op[1]
                        if w_batch:
                            nc.scalar.dma_start(
                                w_sb[:, c * KC : (c + 1) * KC, :], w_d[c]
                            )
                        else:
                            for k in range(KC):
                                nc.scalar.dma_start(
                                    w_sb[:, c * KC + k, :],
                                    w_d[c, :, k * D_OUT : (k + 1) * D_OUT],
                                )

                for ti, (c, t) in enumerate(tile_order):
                    g, lo = tile_src[(c, t)]
                    y_ps = psyp.tile([128, D_OUT], f32)
                    for k in range(KC):
                        nc.tensor.matmul(
                            y_ps[:],
                            g[:, k, lo : lo + 128],
                            w_sb[:, c * KC + k, :],
                            start=(k == 0),
                            stop=(k == KC - 1),
                        )
                    y_sb = ypool.tile([128, D_OUT], y_dt)
                    if copy_split and ti % 2:
                        nc.scalar.copy(y_sb[:], y_ps[:])
                    else:
                        nc.vector.tensor_copy(y_sb[:], y_ps[:])
                    row0 = base[c] + t * 128
                    eng = nc.sync if ti % 2 == 0 else nc.scalar
                    eng.dma_start(y_d[row0 : row0 + 128, :], y_sb[:])
        elif pipelined and loop_reps > 1:
            # software-pipelined timing loop: load[i+2] || compute[i+1] ||
            # store[i]; the all-engine barrier amortizes over `unroll` ticks
            chunk_plan = []
            xo = 0
            for c in range(CPL):
                t0 = 0
                for gc in plan_chunks(
                    n_tiles[c], gather_chunk, first_small, c, last_small
                ):
                    chunk_plan.append((c, t0, gc, xo))
                    xo += KC * gc * 128
                    t0 += gc

            with tc.tile_pool(name="psy", bufs=psum_bufs, space="PSUM") as psyp:

                def st_load(pipe, iv):
                    w_t = pipe.intermediate_tile([128, CPL * KC, D_OUT], bf16)
                    gs = []
                    for c, t0, gc, off in chunk_plan:
                        g = pipe.intermediate_tile([128, KC, gc * 128], bf16)
                        nc.sync.dma_start(g[:], x_d[:, off : off + KC * gc * 128])
                        gs.append(g)
                    for c in range(CPL):
                        nc.scalar.dma_start(
                            w_t[:, c * KC : (c + 1) * KC, :], w_d[c]
                        )
                    return (w_t, *gs)

                def st_compute(pipe, iv, tiles):
                    w_t = tiles[0]
                    gs = tiles[1:]
                    tsrc = {}
                    for gi, (c, t0, gc, off) in enumerate(chunk_plan):
                        for j in range(gc):
                            tsrc[(c, t0 + j)] = (gs[gi], j * 128)
                    ybigs = []
                    ti = 0
                    for c in range(CPL):
                        y_big = pipe.intermediate_tile(
                            [128, n_tiles[c], D_OUT], y_dt
                        )
                        for t in range(n_tiles[c]):
                            g, lo = tsrc[(c, t)]
                            y_ps = psyp.tile([128, D_OUT], f32)
                            for k in range(KC):
                                nc.tensor.matmul(
                                    y_ps[:],
                                    g[:, k, lo : lo + 128],
                                    w_t[:, c * KC + k, :],
                                    start=(k == 0),
                                    stop=(k == KC - 1),
                                )
                            if copy_split and ti % 2:
                                nc.scalar.copy(y_big[:, t, :], y_ps[:])
                            else:
                                nc.vector.tensor_copy(y_big[:, t, :], y_ps[:])
                            ti += 1
                        ybigs.append(y_big)
                    return tuple(ybigs)

                def st_store(pipe, iv, ybigs):
                    for c, y_big in enumerate(ybigs):
                        eng = nc.sync if c % 2 == 0 else nc.scalar
                        eng.dma_start(
                            y_d[base[c] : base[c] + caps[c], :].rearrange(
                                "(t p) n -> p t n", p=128
                            ),
                            y_big[:],
                        )

                tc.For_i_pipelined(
                    [st_load, st_compute, st_store],
                    0,
                    loop_reps,
                    unroll=unroll,
                )
        else:
            _build_sequential = True
        if not (interleave or (pipelined and loop_reps > 1)):
          eff_reps = reps if loop_reps > 1 else 1
          trips = (loop_reps + eff_reps - 1) // eff_reps
          with (
            tc.tile_pool(name="wpool", bufs=wbufs) as wpool,
            tc.tile_pool(name="xin", bufs=gbufs) as gpool,
            tc.tile_pool(name="yout", bufs=ybufs) as ypool,
            tc.tile_pool(name="psy", bufs=psum_bufs, space="PSUM") as psyp,
            tc.For_i(0, trips, 1, staggered_reset=staggered)
            if loop_reps > 1
            else nullcontext(),
          ):
           for _rep in range(eff_reps):
            w_sb = wpool.tile([128, CPL * KC, D_OUT], bf16)
            w_dma = nc.scalar if w_eng == "scalar" else nc.sync
            tile_src = {}  # (c, t) -> (x tile, col offset)
            n_load = 0
            x_off = 0
            if no_loads or detach:
                tiny = gpool.tile([128, 8], bf16)
                nc.sync.dma_start(tiny[:], x_d[:, 0:8])
                for c in range(CPL):
                    g = gpool.tile([128, KC, n_tiles[c] * 128], bf16)
                    nc.vector.memset(g[:, 0, 0:8], 0.0)
                    for t in range(n_tiles[c]):
                        tile_src[(c, t)] = (g, t * 128)
                if not no_compute:
                    nc.vector.memset(w_sb[:, 0, 0:8], 0.0)
            detached = {}
            if warm_mms:
                warm_ps = psyp.tile([128, 64], f32, tag="warmps")

            def warm_kick(src_ap):
                # keep the PE HAM activity window non-idle during the load
                # phase: a ~80ns matmul chained to each arriving chunk
                if warm_mms:
                    nc.tensor.matmul(
                        warm_ps[:],
                        src_ap,
                        src_ap[:, 0:64],
                        start=True,
                        stop=True,
                        skip_group_check=True,
                    )

            for c in range(CPL if not no_loads else 0):

                def load_w(c=c):
                    # W[c] K-chunked SBUF image: [128, KC, D_OUT]
                    if detach:
                        w_dst = wpool.tile([128, KC, D_OUT], bf16, tag="wdump")
                    else:
                        w_dst = w_sb[:, c * KC : (c + 1) * KC, :]
                    if w_batch:
                        w_dma.dma_start(w_dst[:] if detach else w_dst, w_d[c])
                    else:
                        for k in range(KC):
                            w_dma.dma_start(
                                w_dst[:, k, :]
                                if detach
                                else w_sb[:, c * KC + k, :],
                                w_d[c, :, k * D_OUT : (k + 1) * D_OUT],
                            )

                if w_first:
                    load_w()
                t0 = 0
                for gc in plan_chunks(n_tiles[c], gather_chunk, first_small, c, last_small):
                    g = gpool.tile([128, KC, gc * 128], bf16)
                    width = KC * gc * 128
                    x_dma = (
                        (nc.sync if n_load % 2 == 0 else nc.scalar)
                        if load_alt
                        else nc.sync
                    )
                    x_dma.dma_start(g[:], x_d[:, x_off : x_off + width])
                    x_off += width
                    warm_kick(g[:, 0, 0:128])
                    for j in range(gc):
                        if detach:
                            detached[(c, t0 + j)] = (g, j * 128)
                        else:
                            tile_src[(c, t0 + j)] = (g, j * 128)
                    t0 += gc
                    n_load += 1
                if not w_first:
                    load_w()

            # bias is folded in on the host during the scatter-back
            def pick_store(i):
                if store_eng == "alt":
                    return nc.sync if i % 2 == 0 else nc.scalar
                if store_eng == "gpsimd":
                    # Pool engine is otherwise idle: store stalls (waiting on
                    # copies) queue there instead of blocking the next
                    # instance's W/x load issues on the ACT/SP FIFOs
                    return nc.gpsimd
                return nc.scalar if store_eng == "scalar" else nc.sync

            ti = 0
            for c in range(CPL if not no_compute else 0):
                if out_batch:
                    y_big = ypool.tile([128, n_tiles[c], D_OUT], y_dt)
                for t in range(n_tiles[c]):
                    g, lo = tile_src[(c, t)]
                    y_ps = psyp.tile([128, D_OUT], f32)
                    for k in range(KC):
                        nc.tensor.matmul(
                            y_ps[:],
                            g[:, k, lo : lo + 128],
                            w_sb[:, c * KC + k, :],
                            start=(k == 0),
                            stop=(k == KC - 1),
                        )
                    use_act = copy_split and ti % 2

                    def cp(dst, src):
                        if use_act:
                            nc.scalar.copy(dst, src)
                        else:
                            nc.vector.tensor_copy(dst, src)

                    if out_batch:
                        cp(y_big[:, t, :], y_ps[:])
                    else:
                        y_sb = ypool.tile([128, D_OUT], y_dt)
                        cp(y_sb[:], y_ps[:])
                        row0 = base[c] + t * 128
                        if not no_stores:
                            pick_store(ti).dma_start(
                                y_d[row0 : row0 + 128, :], y_sb[:]
                            )
                    ti += 1
                if out_batch and not no_stores:
                    pick_store(c).dma_start(
                        y_d[base[c] : base[c] + caps[c], :].rearrange(
                            "(t p) n -> p t n", p=128
                        ),
                        y_big[:],
                    )

    nc.compile()
    return nc


def _route(cls_np: np.ndarray):
    """Routing: per-class row lists, class->slot assignment, slot capacities.

    Classes are sorted by row count; the 8 largest go to slot 0 (one per
    core), the 8 smallest to slot 1, so the shared per-slot capacity
    (max over cores, rounded up to 128) is minimal.
    """
    order = np.argsort(cls_np, kind="stable")
    counts = np.bincount(cls_np, minlength=C)
    starts = np.zeros(C + 1, dtype=np.int64)
    starts[1:] = np.cumsum(counts)
    rows_per_class = [order[starts[c] : starts[c + 1]] for c in range(C)]

    by_size = sorted(range(C), key=lambda c: -counts[c])
    perm = [0] * C  # global slot g = core*CPL + j -> original class id
    for k in range(NCORES):
        perm[k * CPL] = by_size[k]  # big classes in slot 0
        perm[k * CPL + 1] = by_size[C - 1 - k]  # small classes in slot 1
    cap = [0] * CPL
    for j in range(CPL):
        mx = max(counts[perm[k * CPL + j]] for k in range(NCORES))
        cap[j] = max(128, -(-int(mx) // 128) * 128)
    return rows_per_class, perm, tuple(cap)


# Variant shipped by kernel(); exp.py/bench.py sweep alternatives.
BEST_VARIANT = {
    "gather_chunk": 2,
    "first_small": True,
    "out_batch": True,
    "store_eng": "alt",
    "copy_split": True,
    "psum_bufs": 8,
    "wbufs": 6,
    "gbufs": 30,
    "ybufs": 18,
    "staggered": True,
    "reps": 6,
}


def make_in_maps(x, rows_per_class, perm, caps, W, b, **variant):
    """Per-core input maps matching build_nc(caps, **variant)."""
    import concourse.mybir as mybir

    bf16 = mybir.dt.np(mybir.dt.bfloat16)
    n_tiles = [v // 128 for v in caps]
    gather_chunk = variant.get("gather_chunk", 3)
    first_small = variant.get("first_small", True)
    last_small = variant.get("last_small", False)
    interleave = variant.get("interleave", False)
    x_bf = np.ascontiguousarray(np.asarray(x, dtype=np.float32).astype(bf16))
    W_bf = np.asarray(W, dtype=np.float32).astype(bf16)

    def img(blk, gc):
        # SBUF image [128, KC, gc*128]: g[p,kk,r] = blk[r, kk*128+p]
        return (
            blk.reshape(gc * 128, KC, 128)
            .transpose(2, 1, 0)
            .reshape(128, KC * gc * 128)
        )

    in_maps = []
    for k in range(NCORES):
        xcs = []
        wls = []
        for j in range(CPL):
            c = perm[k * CPL + j]
            rows = rows_per_class[c]
            idx = np.zeros(caps[j], dtype=np.int64)
            idx[: len(rows)] = rows
            xcs.append(x_bf[idx])  # [caps[j], D_IN]
            # W image [128, KC*D_OUT]: w[p, kk*D+n] = W[c, kk*128+p, n]
            wls.append(
                W_bf[c]
                .reshape(KC, 128, D_OUT)
                .transpose(1, 0, 2)
                .reshape(128, KC * D_OUT)
            )
        cols = []
        if interleave:
            for op in plan_interleave(n_tiles)[0]:
                if op[0] != "x":
                    continue
                _, j, t0, gc = op
                cols.append(img(xcs[j][t0 * 128 : (t0 + gc) * 128], gc))
        else:
            for j in range(CPL):
                t0 = 0
                for gc in plan_chunks(
                    n_tiles[j], gather_chunk, first_small, j, last_small
                ):
                    cols.append(img(xcs[j][t0 * 128 : (t0 + gc) * 128], gc))
                    t0 += gc
        in_maps.append(
            {
                "xp": np.ascontiguousarray(np.concatenate(cols, axis=1)),
                "wl": np.ascontiguousarray(np.stack(wls)),
            }
        )
    return in_maps


def kernel(x, cls, W, b):
    from concourse.bass_utils import run_bass_kernel_spmd

    global LAST_RESULT
    cls_np = np.asarray(cls).astype(np.int64).ravel()

    rows_per_class, perm, caps = _route(cls_np)

    in_maps = make_in_maps(x, rows_per_class, perm, caps, W, b, **BEST_VARIANT)
    nc = build_nc(caps, **BEST_VARIANT)
    res = run_bass_kernel_spmd(
        nc,
        in_maps,
        core_ids=list(range(NCORES)),
        trace=TRACE,
        trace_cores=list(range(NCORES)) if TRACE else None,
    )
    LAST_RESULT = res

    b_np = np.asarray(b, dtype=np.float32)
    base = [sum(caps[:j]) for j in range(CPL)]
    out = np.empty((B, D_OUT), dtype=np.float32)
    for k in range(NCORES):
        y = np.asarray(res.results[k]["y"]).astype(np.float32)
        for j in range(CPL):
            c = perm[k * CPL + j]
            rows = rows_per_class[c]
            out[rows] = y[base[j] : base[j] + len(rows)] + b_np[c]
    return out


# revision 68
# speedup vs baseline: 1.7882x; 1.7514x over previous
"""Class-conditional linear dispatch (MoE routing) on 8 trn2 NeuronCores.

y[i] = x[i] @ W[cls[i]] + b[cls[i]]   with B=8192, D=512, C=16 classes.

Strategy: expert-parallel with host-side dispatch. The host computes the
routing from `cls` (argsort by class), assigns two classes per core --
pairing large classes with small ones so the per-core tile count is
minimal -- converts x/W to bf16, and pre-packs each core's rows as the
exact SBUF images the kernel wants (K-major, chunked), so every device
load is a plain HWDGE DMA with 128 contiguous per-partition runs.

On device, per 128-row tile: 4 accumulating bf16 matmuls against the
class's K-chunked weights (f32 PSUM), then a DVE/ACT copy (alternating,
so the two engines work in parallel) casts to bf16 into a per-class
output block that is stored as one large DMA per class (alternating
SP/ACT rings). The host scatters the per-core bf16 outputs back to
original row order, adding the (f32) bias during the scatter.

The For_i timing loop uses staggered_reset and emits `reps` independent
problem instances per loop body (loop_reps keeps meaning instances:
trip count = ceil(loop_reps/reps)). The per-iteration all-engine
barrier amortizes over the instances and the Tile scheduler overlaps
instance i+1's loads with instance i's compute inside one body --
measured 17.5us -> 11.7us per instance going from reps=1 to reps=6.
The design also minimizes total DMA bytes (all-bf16: 2.125 MB in +
1.125 MB out per core), DMA count (large contiguous host-prepacked
images), and PE work (36 matmuls; classes paired big+small so per-core
capacity is 640+512 rows instead of 640+640).
"""

import sys

import numpy as np

_TRN_REPO = "/opt/trn_rl_repo"
if _TRN_REPO not in sys.path:
    sys.path.insert(0, _TRN_REPO)

B, D_IN, D_OUT, C, NCORES = 8192, 512, 512, 16, 8
CPL = C // NCORES  # class slots per core
KC = D_IN // 128  # contraction chunks of 128

# Set by callers that want profiling; results stashed in LAST_RESULT.
TRACE = False
LAST_RESULT = None


def plan_chunks(n_tiles, gather_chunk, first_small, c, last_small=False):
    """Row-tile chunk sizes for class-slot c's loads (host & device agree)."""
    chunks, rem = [], n_tiles
    tail = 1 if last_small and c == CPL - 1 and n_tiles > 1 else 0
    rem -= tail
    if first_small and c == 0 and rem > 1:
        chunks.append(1)
        rem -= 1
    while rem > 0:
        gc = min(gather_chunk, rem)
        chunks.append(gc)
        rem -= gc
    if tail:
        chunks.append(1)
    return chunks


def plan_interleave(n_tiles):
    """Class-interleaved load + compute order (host & device agree).

    Loads: x c0 (2 tiles), W c0, x c1 (2 tiles), W c1, then alternating
    2-tile x chunks. Compute consumes tiles in the same arrival order, so
    the PE starts after ~768KB and never starves while later chunks and
    the stores share the remaining DMA stream.
    """
    load_ops = []  # ("x", c, t0, gc) | ("w", c)
    pos = [0] * CPL
    for c in range(CPL):
        gc = min(2, n_tiles[c])
        load_ops.append(("x", c, 0, gc))
        load_ops.append(("w", c))
        pos[c] = gc
    turn = 0
    while any(pos[c] < n_tiles[c] for c in range(CPL)):
        c = turn % CPL
        if pos[c] < n_tiles[c]:
            gc = min(2, n_tiles[c] - pos[c])
            load_ops.append(("x", c, pos[c], gc))
            pos[c] += gc
        turn += 1
    tile_order = []
    tp = [0] * CPL
    turn = 0
    while any(tp[c] < n_tiles[c] for c in range(CPL)):
        c = turn % CPL
        n = 0
        while tp[c] < n_tiles[c] and n < 2:
            tile_order.append((c, tp[c]))
            tp[c] += 1
            n += 1
        turn += 1
    return load_ops, tile_order


def build_nc(
    caps,
    *,
    gather_chunk: int = 3,
    first_small: bool = True,
    last_small: bool = False,
    y_f32: bool = False,
    w_eng: str = "scalar",
    w_batch: bool = True,
    w_first: bool = False,
    load_alt: bool = False,
    store_eng: str = "sync",
    out_batch: bool = False,
    copy_split: bool = False,
    gbufs: int = 4,
    wbufs: int = 1,
    psum_bufs: int = 3,
    ybufs: int = 4,
    staggered: bool = False,
    warm_mms: bool = False,
    pipelined: bool = False,
    interleave: bool = False,
    unroll: int = 4,
    reps: int = 1,
    loop_reps: int = 1,
    no_compute: bool = False,
    no_loads: bool = False,
    no_stores: bool = False,
    detach: bool = False,
):
    """Per-core Bass program. caps = rows capacity per class slot
    (each a multiple of 128); int means uniform."""
    import concourse.bacc as bacc
    import concourse.mybir as mybir
    from concourse import tile

    f32 = mybir.dt.float32
    bf16 = mybir.dt.bfloat16
    if isinstance(caps, int):
        caps = (caps,) * CPL
    caps = tuple(int(v) for v in caps)
    n_tiles = [v // 128 for v in caps]
    base = [sum(caps[:c]) for c in range(CPL)]
    r_cap = sum(caps)
    x_cols = KC * r_cap

    nc = bacc.Bacc("TRN2", target_bir_lowering=False, debug=False)
    x_d = nc.dram_tensor("xp", [128, x_cols], bf16, kind="ExternalInput")
    w_d = nc.dram_tensor("wl", [CPL, 128, KC * D_OUT], bf16, kind="ExternalInput")
    y_dt = f32 if y_f32 else bf16
    y_d = nc.dram_tensor("y", [r_cap, D_OUT], y_dt, kind="ExternalOutput")

    with tile.TileContext(nc) as tc:
        from contextlib import ExitStack, nullcontext

        if interleave:
            load_ops, tile_order = plan_interleave(n_tiles)
            with (
                tc.tile_pool(name="wpool", bufs=wbufs) as wpool,
                tc.tile_pool(name="xin", bufs=gbufs) as gpool,
                tc.tile_pool(name="yout", bufs=ybufs) as ypool,
                tc.tile_pool(name="psy", bufs=psum_bufs, space="PSUM") as psyp,
                tc.For_i(0, loop_reps, 1, staggered_reset=staggered)
                if loop_reps > 1
                else nullcontext(),
            ):
                w_sb = wpool.tile([128, CPL * KC, D_OUT], bf16)
                tile_src = {}
                x_off = 0
                for op in load_ops:
                    if op[0] == "x":
                        _, c, t0, gc = op
                        g = gpool.tile([128, KC, gc * 128], bf16)
                        width = KC * gc * 128
                        nc.sync.dma_start(g[:], x_d[:, x_off : x_off + width])
                        x_off += width
                        for j in range(gc):
                            tile_src[(c, t0 + j)] = (g, j * 128)
                    else:
                        c = op[1]
                        if w_batch:
                            nc.scalar.dma_start(
                                w_sb[:, c * KC : (c + 1) * KC, :], w_d[c]
                            )
                        else:
                            for k in range(KC):
                                nc.scalar.dma_start(
                                    w_sb[:, c * KC + k, :],
                                    w_d[c, :, k * D_OUT : (k + 1) * D_OUT],
                                )

                for ti, (c, t) in enumerate(tile_order):
                    g, lo = tile_src[(c, t)]
                    y_ps = psyp.tile([128, D_OUT], f32)
                    for k in range(KC):
                        nc.tensor.matmul(
                            y_ps[:],
                            g[:, k, lo : lo + 128],
                            w_sb[:, c * KC + k, :],
                            start=(k == 0),
                            stop=(k == KC - 1),
                        )
                    y_sb = ypool.tile([128, D_OUT], y_dt)
                    if copy_split and ti % 2:
                        nc.scalar.copy(y_sb[:], y_ps[:])
                    else:
                        nc.vector.tensor_copy(y_sb[:], y_ps[:])
                    row0 = base[c] + t * 128
                    eng = nc.sync if ti % 2 == 0 else nc.scalar
                    eng.dma_start(y_d[row0 : row0 + 128, :], y_sb[:])
        elif pipelined and loop_reps > 1:
            # software-pipelined timing loop: load[i+2] || compute[i+1] ||
            # store[i]; the all-engine barrier amortizes over `unroll` ticks
            chunk_plan = []
            xo = 0
            for c in range(CPL):
                t0 = 0
                for gc in plan_chunks(
                    n_tiles[c], gather_chunk, first_small, c, last_small
                ):
                    chunk_plan.append((c, t0, gc, xo))
                    xo += KC * gc * 128
                    t0 += gc

            with tc.tile_pool(name="psy", bufs=psum_bufs, space="PSUM") as psyp:

                def st_load(pipe, iv):
                    w_t = pipe.intermediate_tile([128, CPL * KC, D_OUT], bf16)
                    gs = []
                    for c, t0, gc, off in chunk_plan:
                        g = pipe.intermediate_tile([128, KC, gc * 128], bf16)
                        nc.sync.dma_start(g[:], x_d[:, off : off + KC * gc * 128])
                        gs.append(g)
                    for c in range(CPL):
                        nc.scalar.dma_start(
                            w_t[:, c * KC : (c + 1) * KC, :], w_d[c]
                        )
                    return (w_t, *gs)

                def st_compute(pipe, iv, tiles):
                    w_t = tiles[0]
                    gs = tiles[1:]
                    tsrc = {}
                    for gi, (c, t0, gc, off) in enumerate(chunk_plan):
                        for j in range(gc):
                            tsrc[(c, t0 + j)] = (gs[gi], j * 128)
                    ybigs = []
                    ti = 0
                    for c in range(CPL):
                        y_big = pipe.intermediate_tile(
                            [128, n_tiles[c], D_OUT], y_dt
                        )
                        for t in range(n_tiles[c]):
                            g, lo = tsrc[(c, t)]
                            y_ps = psyp.tile([128, D_OUT], f32)
                            for k in range(KC):
                                nc.tensor.matmul(
                                    y_ps[:],
                                    g[:, k, lo : lo + 128],
                                    w_t[:, c * KC + k, :],
                                    start=(k == 0),
                                    stop=(k == KC - 1),
                                )
                            if copy_split and ti % 2:
                                nc.scalar.copy(y_big[:, t, :], y_ps[:])
                            else:
                                nc.vector.tensor_copy(y_big[:, t, :], y_ps[:])
                            ti += 1
                        ybigs.append(y_big)
                    return tuple(ybigs)

                def st_store(pipe, iv, ybigs):
                    for c, y_big in enumerate(ybigs):
                        eng = nc.sync if c % 2 == 0 else nc.scalar
                        eng.dma_start(
                            y_d[base[c] : base[c] + caps[c], :].rearrange(
                                "(t p) n -> p t n", p=128
                            ),
                            y_big[:],
                        )

                tc.For_i_pipelined(
                    [st_load, st_compute, st_store],
                    0,
                    loop_reps,
                    unroll=unroll,
                )
        else:
            _build_sequential = True
        if not (interleave or (pipelined and loop_reps > 1)):
          eff_reps = reps if loop_reps > 1 else 1
          trips = (loop_reps + eff_reps - 1) // eff_reps
          with (
            tc.tile_pool(name="wpool", bufs=wbufs) as wpool,
            tc.tile_pool(name="xin", bufs=gbufs) as gpool,
            tc.tile_pool(name="yout", bufs=ybufs) as ypool,
            tc.tile_pool(name="psy", bufs=psum_bufs, space="PSUM") as psyp,
            tc.For_i(0, trips, 1, staggered_reset=staggered)
            if loop_reps > 1
            else nullcontext(),
          ):
           for _rep in range(eff_reps):
            w_sb = wpool.tile([128, CPL * KC, D_OUT], bf16)
            w_dma = nc.scalar if w_eng == "scalar" else nc.sync
            tile_src = {}  # (c, t) -> (x tile, col offset)
            n_load = 0
            x_off = 0
            if no_loads or detach:
                tiny = gpool.tile([128, 8], bf16)
                nc.sync.dma_start(tiny[:], x_d[:, 0:8])
                for c in range(CPL):
                    g = gpool.tile([128, KC, n_tiles[c] * 128], bf16)
                    nc.vector.memset(g[:, 0, 0:8], 0.0)
                    for t in range(n_tiles[c]):
                        tile_src[(c, t)] = (g, t * 128)
                if not no_compute:
                    nc.vector.memset(w_sb[:, 0, 0:8], 0.0)
            detached = {}
            if warm_mms:
                warm_ps = psyp.tile([128, 64], f32, tag="warmps")

            def warm_kick(src_ap):
                # keep the PE HAM activity window non-idle during the load
                # phase: a ~80ns matmul chained to each arriving chunk
                if warm_mms:
                    nc.tensor.matmul(
                        warm_ps[:],
                        src_ap,
                        src_ap[:, 0:64],
                        start=True,
                        stop=True,
                        skip_group_check=True,
                    )

            for c in range(CPL if not no_loads else 0):

                def load_w(c=c):
                    # W[c] K-chunked SBUF image: [128, KC, D_OUT]
                    if detach:
                        w_dst = wpool.tile([128, KC, D_OUT], bf16, tag="wdump")
                    else:
                        w_dst = w_sb[:, c * KC : (c + 1) * KC, :]
                    if w_batch:
                        w_dma.dma_start(w_dst[:] if detach else w_dst, w_d[c])
                    else:
                        for k in range(KC):
                            w_dma.dma_start(
                                w_dst[:, k, :]
                                if detach
                                else w_sb[:, c * KC + k, :],
                                w_d[c, :, k * D_OUT : (k + 1) * D_OUT],
                            )

                if w_first:
                    load_w()
                t0 = 0
                for gc in plan_chunks(n_tiles[c], gather_chunk, first_small, c, last_small):
                    g = gpool.tile([128, KC, gc * 128], bf16)
                    width = KC * gc * 128
                    x_dma = (
                        (nc.sync if n_load % 2 == 0 else nc.scalar)
                        if load_alt
                        else nc.sync
                    )
                    x_dma.dma_start(g[:], x_d[:, x_off : x_off + width])
                    x_off += width
                    warm_kick(g[:, 0, 0:128])
                    for j in range(gc):
                        if detach:
                            detached[(c, t0 + j)] = (g, j * 128)
                        else:
                            tile_src[(c, t0 + j)] = (g, j * 128)
                    t0 += gc
                    n_load += 1
                if not w_first:
                    load_w()

            # bias is folded in on the host during the scatter-back
            def pick_store(i):
                if store_eng == "alt":
                    return nc.sync if i % 2 == 0 else nc.scalar
                if store_eng == "gpsimd":
                    # Pool engine is otherwise idle: store stalls (waiting on
                    # copies) queue there instead of blocking the next
                    # instance's W/x load issues on the ACT/SP FIFOs
                    return nc.gpsimd
                return nc.scalar if store_eng == "scalar" else nc.sync

            ti = 0
            for c in range(CPL if not no_compute else 0):
                if out_batch:
                    y_big = ypool.tile([128, n_tiles[c], D_OUT], y_dt)
                for t in range(n_tiles[c]):
                    g, lo = tile_src[(c, t)]
                    y_ps = psyp.tile([128, D_OUT], f32)
                    for k in range(KC):
                        nc.tensor.matmul(
                            y_ps[:],
                            g[:, k, lo : lo + 128],
                            w_sb[:, c * KC + k, :],
                            start=(k == 0),
                            stop=(k == KC - 1),
                        )
                    use_act = copy_split and ti % 2

                    def cp(dst, src):
                        if use_act:
                            nc.scalar.copy(dst, src)
                        else:
                            nc.vector.tensor_copy(dst, src)

                    if out_batch:
                        cp(y_big[:, t, :], y_ps[:])
                    else:
                        y_sb = ypool.tile([128, D_OUT], y_dt)
                        cp(y_sb[:], y_ps[:])
                        row0 = base[c] + t * 128
                        if not no_stores:
                            pick_store(ti).dma_start(
                                y_d[row0 : row0 + 128, :], y_sb[:]
                            )
                    ti += 1
                if out_batch and not no_stores:
                    pick_store(c).dma_start(
                        y_d[base[c] : base[c] + caps[c], :].rearrange(
                            "(t p) n -> p t n", p=128
                        ),
                        y_big[:],
                    )

    nc.compile()
    return nc


def _route(cls_np: np.ndarray):
    """Routing: per-class row lists, class->slot assignment, slot capacities.

    Classes are sorted by row count; the 8 largest go to slot 0 (one per
    core), the 8 smallest to slot 1, so the shared per-slot capacity
    (max over cores, rounded up to 128) is minimal.
    """
    order = np.argsort(cls_np, kind="stable")
    counts = np.bincount(cls_np, minlength=C)
    starts = np.zeros(C + 1, dtype=np.int64)
    starts[1:] = np.cumsum(counts)
    rows_per_class = [order[starts[c] : starts[c + 1]] for c in range(C)]

    by_size = sorted(range(C), key=lambda c: -counts[c])
    perm = [0] * C  # global slot g = core*CPL + j -> original class id
    for k in range(NCORES):
        perm[k * CPL] = by_size[k]  # big classes in slot 0
        perm[k * CPL + 1] = by_size[C - 1 - k]  # small classes in slot 1
    cap = [0] * CPL
    for j in range(CPL):
        mx = max(counts[perm[k * CPL + j]] for k in range(NCORES))
        cap[j] = max(128, -(-int(mx) // 128) * 128)
    return rows_per_class, perm, tuple(cap)


# Variant shipped by kernel(); exp.py/bench.py sweep alternatives.
BEST_VARIANT = {
    "gather_chunk": 2,
    "first_small": False,
    "out_batch": True,
    "store_eng": "alt",
    "copy_split": True,
    "psum_bufs": 8,
    "wbufs": 6,
    "gbufs": 30,
    "ybufs": 18,
    "staggered": True,
    "reps": 6,
}


def make_in_maps(x, rows_per_class, perm, caps, W, b, **variant):
    """Per-core input maps matching build_nc(caps, **variant)."""
    import concourse.mybir as mybir

    bf16 = mybir.dt.np(mybir.dt.bfloat16)
    n_tiles = [v // 128 for v in caps]
    gather_chunk = variant.get("gather_chunk", 3)
    first_small = variant.get("first_small", True)
    last_small = variant.get("last_small", False)
    interleave = variant.get("interleave", False)
    x_bf = np.ascontiguousarray(np.asarray(x, dtype=np.float32).astype(bf16))
    W_bf = np.asarray(W, dtype=np.float32).astype(bf16)

    def img(blk, gc):
        # SBUF image [128, KC, gc*128]: g[p,kk,r] = blk[r, kk*128+p]
        return (
            blk.reshape(gc * 128, KC, 128)
            .transpose(2, 1, 0)
            .reshape(128, KC * gc * 128)
        )

    in_maps = []
    for k in range(NCORES):
        xcs = []
        wls = []
        for j in range(CPL):
            c = perm[k * CPL + j]
            rows = rows_per_class[c]
            idx = np.zeros(caps[j], dtype=np.int64)
            idx[: len(rows)] = rows
            xcs.append(x_bf[idx])  # [caps[j], D_IN]
            # W image [128, KC*D_OUT]: w[p, kk*D+n] = W[c, kk*128+p, n]
            wls.append(
                W_bf[c]
                .reshape(KC, 128, D_OUT)
                .transpose(1, 0, 2)
                .reshape(128, KC * D_OUT)
            )
        cols = []
        if interleave:
            for op in plan_interleave(n_tiles)[0]:
                if op[0] != "x":
                    continue
                _, j, t0, gc = op
                cols.append(img(xcs[j][t0 * 128 : (t0 + gc) * 128], gc))
        else:
            for j in range(CPL):
                t0 = 0
                for gc in plan_chunks(
                    n_tiles[j], gather_chunk, first_small, j, last_small
                ):
                    cols.append(img(xcs[j][t0 * 128 : (t0 + gc) * 128], gc))
                    t0 += gc
        in_maps.append(
            {
                "xp": np.ascontiguousarray(np.concatenate(cols, axis=1)),
                "wl": np.ascontiguousarray(np.stack(wls)),
            }
        )
    return in_maps


def kernel(x, cls, W, b):
    from concourse.bass_utils import run_bass_kernel_spmd

    global LAST_RESULT
    cls_np = np.asarray(cls).astype(np.int64).ravel()

    rows_per_class, perm, caps = _route(cls_np)

    in_maps = make_in_maps(x, rows_per_class, perm, caps, W, b, **BEST_VARIANT)
    nc = build_nc(caps, **BEST_VARIANT)
    res = run_bass_kernel_spmd(
        nc,
        in_maps,
        core_ids=list(range(NCORES)),
        trace=TRACE,
        trace_cores=list(range(NCORES)) if TRACE else None,
    )
    LAST_RESULT = res

    b_np = np.asarray(b, dtype=np.float32)
    base = [sum(caps[:j]) for j in range(CPL)]
    out = np.empty((B, D_OUT), dtype=np.float32)
    for k in range(NCORES):
        y = np.asarray(res.results[k]["y"]).astype(np.float32)
        for j in range(CPL):
            c = perm[k * CPL + j]
            rows = rows_per_class[c]
            out[rows] = y[base[j] : base[j] + len(rows)] + b_np[c]
    return out
